# revision 34
# baseline (speedup 1.0000x reference)
"""Trainium2 Bass kernel for gated single-head attention (B=4, L=2048, E=512, D=64).

Sharding: data-parallel over 8 cores; core c handles batch b=c//2, query-row
half h=c%2 (1024 query rows). k/v are computed per-core for the full batch
(duplicated across the pair) since softmax needs all 2048 keys.

Math restructuring (validated in numpy against the jax reference, ~4.8e-3 rel):
  - q,k are L2-normalized so scores lie in [-1/8, 1/8]: softmax needs no
    max-subtraction; exp() is applied directly to the transposed score tile.
  - softmax denominator Z is skipped entirely: rmsnorm is invariant to a
    per-row uniform scale (only the eps semantics shift, ~1e-5 effect).
  - rsqrt computed as exp(-0.5*ln(x)) so only two ACT table sets are used
    (sigmoid_and_others + natural_log_exp_and_others).
  - g_rms is folded into Wo on the host; silu(x) = x*sigmoid(x) on DVE.
Compute dtype is bf16 (PE runs 4x slower on f32), accumulation f32 in PSUM.
"""

import os
import sys

import numpy as np

try:
    import concourse.bass as bass
except ImportError:  # staged container path
    sys.path.insert(0, "/opt/trn_rl_repo")
    import concourse.bass as bass

import ml_dtypes
from contextlib import ExitStack

import concourse.bacc as bacc
import concourse.tile as tile
from concourse import mybir
from concourse.bass_utils import run_bass_kernel_spmd
from concourse.masks import make_identity

BF16 = ml_dtypes.bfloat16
F32 = mybir.dt.float32
BF = mybir.dt.bfloat16
AF = mybir.ActivationFunctionType
ALU = mybir.AluOpType

B, L, E, D = 4, 2048, 512, 64
NCORES = 8
R = L // 2          # 1024 query rows per core
RT = R // 128       # 8 query m-tiles per core
KT = L // 128       # 16 kv m-tiles per core
EC = E // 128       # 4 contraction chunks
EPS_RMS = 1e-6
EPS_L2 = 1e-24

LAST = None  # BassKernelResults of the most recent run (for test harness)


def _build(has_bias):
    """Build the per-core SPMD program. has_bias: dict of bool flags."""
    nc = bacc.Bacc(
        "TRN2",
        target_bir_lowering=False,
        debug=False,
        enable_asserts=False,
        num_devices=NCORES,
    )

    hq_d = nc.dram_tensor("hq", [R, E], F32, kind="ExternalInput")
    hk_d = nc.dram_tensor("hk", [L, E], F32, kind="ExternalInput")
    hv_d = nc.dram_tensor("hv", [L, E], F32, kind="ExternalInput")
    hs_d = nc.dram_tensor("hs", [R, E], F32, kind="ExternalInput")
    wq_d = nc.dram_tensor("wq", [E, D], BF, kind="ExternalInput")
    wk_d = nc.dram_tensor("wk", [E, D], BF, kind="ExternalInput")
    wvb_d = nc.dram_tensor("wvb", [E, 2 * D], BF, kind="ExternalInput")
    wa1_d = nc.dram_tensor("wa1", [E, 32], BF, kind="ExternalInput")
    ws1_d = nc.dram_tensor("ws1", [E, 32], BF, kind="ExternalInput")
    wa2_d = nc.dram_tensor("wa2", [32, D], BF, kind="ExternalInput")
    ws2_d = nc.dram_tensor("ws2", [32, D], BF, kind="ExternalInput")
    wo_d = nc.dram_tensor("wo", [D, D], BF, kind="ExternalInput")
    bias_d = {}
    for name, n in [("bq", D), ("bk", D), ("bvb", 2 * D), ("ba1", 32),
                    ("ba2", D), ("bs1", 32), ("bs2", D), ("bo", D)]:
        if has_bias[name]:
            bias_d[name] = nc.dram_tensor(name, [1, n], BF, kind="ExternalInput")
    out_d = nc.dram_tensor("out", [R, D], F32, kind="ExternalOutput")

    with tile.TileContext(nc) as tc, ExitStack() as ctx:
        consts = ctx.enter_context(tc.tile_pool(name="consts", bufs=1))
        persist = ctx.enter_context(tc.tile_pool(name="persist", bufs=1))

        ident = consts.tile([128, 128], BF)
        make_identity(nc, ident)
        ones64 = consts.tile([64, 1], BF)
        nc.vector.memset(ones64, 1.0)
        ones1 = consts.tile([1, 1], BF)
        nc.vector.memset(ones1, 1.0)
        onec = consts.tile([128, D], BF)
        nc.vector.memset(onec, 1.0)
        eps_rms128 = consts.tile([128, 1], F32)
        nc.vector.memset(eps_rms128, EPS_RMS)
        eps_l2 = consts.tile([128, 1], F32)
        nc.vector.memset(eps_l2, EPS_L2)
        any_bias = any(has_bias.values())
        if any_bias:
            ones_row = consts.tile([1, 512], BF)
            nc.vector.memset(ones_row, 1.0)

        def load_w(d, n, nm):
            t = consts.tile([128, EC, n], BF, name=nm)
            nc.sync.dma_start(out=t, in_=d.ap().rearrange("(c p) n -> p c n", p=128))
            return t

        wq = load_w(wq_d, D, "wq_sb")
        wk = load_w(wk_d, D, "wk_sb")
        wvb = load_w(wvb_d, 2 * D, "wvb_sb")
        wa1 = load_w(wa1_d, 32, "wa1_sb")
        ws1 = load_w(ws1_d, 32, "ws1_sb")
        wa2 = consts.tile([32, D], BF)
        nc.sync.dma_start(out=wa2, in_=wa2_d.ap())
        ws2 = consts.tile([32, D], BF)
        nc.sync.dma_start(out=ws2, in_=ws2_d.ap())
        wo = consts.tile([64, D], BF)
        nc.sync.dma_start(out=wo, in_=wo_d.ap())
        bias_sb = {}
        for name, t in bias_d.items():
            n = t.shape[1]
            bt = consts.tile([1, n], BF, name=f"{name}_sb")
            nc.sync.dma_start(out=bt, in_=t.ap())
            bias_sb[name] = bt

        def bias_mm(psum, name, cols=None):
            """Add per-column bias b[1, n] to psum accumulation via K=1 matmul."""
            if name not in bias_sb:
                return False
            b = bias_sb[name]
            if cols is not None:
                b = b[:, cols[0]:cols[1]]
            nc.tensor.matmul(psum, ones_row[:, : psum.shape[0]], b.rearrange("o n -> o n"),
                             start=False, stop=True)
            return True

        def biasT_mm(psum, name):
            """Add per-row bias (transposed layouts): psum[r, m] += b[r]."""
            if name not in bias_sb:
                return False
            nc.tensor.matmul(psum, bias_sb[name], ones_row[:, : psum.free_size()],
                             start=False, stop=True)
            return True

        # persistent SBUF tensors
        q_full = persist.tile([128, RT, D], F32)
        k_full = persist.tile([128, KT, D], F32)
        ss_q = persist.tile([128, RT], F32)
        ss_k = persist.tile([128, KT], F32)
        rs_q = persist.tile([128, RT], F32)
        rs_k = persist.tile([128, KT], F32)
        ms_cols = persist.tile([128, RT], F32)
        rs_cols = persist.tile([128, RT], F32)
        qn = persist.tile([128, RT, D], BF)
        kn = persist.tile([128, KT, D], BF)
        qT2 = persist.tile([128, R], BF)   # rows 0:64 = qT, 64:128 = copy
        kT = persist.tile([64, L], BF)
        kT2 = persist.tile([128, KT // 2, 128], BF)  # even jt rows 0:64, odd 64:128
        vb_tanh = persist.tile([128, KT, 2 * D], BF)  # tanh(v'), tanh(b')
        v_full = persist.tile([128, KT, D], BF)
        v1 = persist.tile([128, KT, D], BF)
        a1T = persist.tile([32, L], BF)
        s1T = persist.tile([32, R], BF)
        tsc = persist.tile([64, R], BF)   # tanh of halved shortcut pre-act
        eT = persist.tile([128, KT, R], BF)
        out_sb = persist.tile([128, RT, D], F32)

        evac_ct = [0]

        def evac(dst, src):
            """Copy PSUM->SBUF alternating between ACT and DVE."""
            evac_ct[0] += 1
            if evac_ct[0] % 2:
                nc.scalar.copy(dst, src)
            else:
                nc.vector.tensor_copy(dst, src)

        # ---------------- Phase A: load, transpose, project ----------------
        with tc.tile_pool(name="loadp", bufs=3) as loadp, \
             tc.tile_pool(name="xtp", bufs=2) as xtp, \
             tc.tile_pool(name="sigp", bufs=3) as sigp, \
             tc.tile_pool(name="ps_tp", bufs=2, space="PSUM") as ps_tp, \
             tc.tile_pool(name="ps_proj", bufs=2, space="PSUM") as ps_proj, \
             tc.tile_pool(name="ps_pt", bufs=2, space="PSUM") as ps_pt, \
             tc.tile_pool(name="ps_mini", bufs=2, space="PSUM") as ps_mini:

            def load_and_transpose(src_d, nblk, blk):
                src = src_d.ap().rearrange("(b t p) e -> b p t e", p=128, t=4)
                nat = loadp.tile([128, 4, E], BF, tag="nat", name="nat")
                nc.gpsimd.dma_start(out=nat, in_=src[blk])
                xt = xtp.tile([128, EC, 512], BF, tag="xt", name="xt")
                for cc in range(EC // 2):
                    # two e-chunks of transposes share one psum bank (bf16)
                    ps = ps_tp.tile([128, 2, 512], BF, tag="tp", name="ps_t")
                    for ci in range(2):
                        c = 2 * cc + ci
                        for t in range(4):
                            nc.tensor.transpose(
                                ps[:, ci, t * 128:(t + 1) * 128],
                                nat[:, t, c * 128:(c + 1) * 128],
                                ident,
                            )
                    nc.vector.tensor_copy(xt[:, 2 * cc:2 * cc + 2, :], ps)
                return xt

            def process(src_d, nblk, kind):
                for blk in range(nblk):
                    xt = load_and_transpose(src_d, nblk, blk)

                    if kind == "v":
                        for u in range(2):
                            jt = blk * 4 + 2 * u
                            # two m-tiles' [v|beta] groups in one psum bank
                            pvb = ps_proj.tile([128, 2, 2 * D], F32, tag="proj", name="pvb")
                            for h in range(2):
                                for c in range(EC):
                                    nc.tensor.matmul(
                                        pvb[:, h, :],
                                        xt[:, c, (2 * u + h) * 128:(2 * u + h + 1) * 128],
                                        wvb[:, c, :], start=(c == 0),
                                        stop=(c == EC - 1 and not has_bias["bvb"]))
                                bias_mm(pvb[:, h, :], "bvb")
                            nc.scalar.activation(vb_tanh[:, jt:jt + 2, :], pvb, AF.Tanh)
                            # v = x'*(tanh(x')+1) with x' = (hv@Wv + bv)/2
                            nc.vector.scalar_tensor_tensor(
                                out=v_full[:, jt:jt + 2, :],
                                in0=vb_tanh[:, jt:jt + 2, :D], scalar=1.0,
                                in1=pvb[:, :, :D], op0=ALU.add, op1=ALU.mult)
                        # a1T for this block
                        pa1 = ps_pt.tile([32, 512], F32, tag="pt", name="pa1")
                        for c in range(EC):
                            nc.tensor.matmul(pa1, wa1[:, c, :], xt[:, c, :],
                                             start=(c == 0),
                                             stop=(c == EC - 1 and not has_bias["ba1"]))
                        biasT_mm(pa1, "ba1")
                        evac(a1T[:, blk * 512:(blk + 1) * 512], pa1)
                        # alpha for this block's 4 m-tiles, then v1 = v*alpha+beta
                        # (scaled by 2: v1' = v*(ta+1) + (tb+1), cancels in rmsnorm)
                        pa2 = ps_proj.tile([128, 4, D], F32, tag="proj", name="pa2")
                        for h in range(4):
                            jt = blk * 4 + h
                            nc.tensor.matmul(pa2[:, h, :],
                                             a1T[:, jt * 128:(jt + 1) * 128], wa2,
                                             start=True, stop=not has_bias["ba2"])
                            if has_bias["ba2"]:
                                bias_mm(pa2[:, h, :], "ba2")
                        alf = sigp.tile([128, 4, D], BF, tag="sig", name="alf")
                        nc.scalar.activation(alf, pa2, AF.Tanh)
                        for h in range(4):
                            jt = blk * 4 + h
                            t1 = sigp.tile([128, D], BF, tag="t1", name="t1")
                            nc.vector.scalar_tensor_tensor(
                                out=t1, in0=alf[:, h, :], scalar=1.0,
                                in1=v_full[:, jt, :], op0=ALU.add, op1=ALU.mult)
                            c1 = sigp.tile([128, D], BF, tag="c1", name="c1")
                            nc.gpsimd.tensor_add(c1, vb_tanh[:, jt, D:], onec)
                            nc.gpsimd.tensor_add(v1[:, jt, :], t1, c1)
                    elif kind == "s":
                        ps1 = ps_pt.tile([32, 512], F32, tag="pt", name="ps1")
                        for c in range(EC):
                            nc.tensor.matmul(ps1, ws1[:, c, :], xt[:, c, :],
                                             start=(c == 0),
                                             stop=(c == EC - 1 and not has_bias["bs1"]))
                        biasT_mm(ps1, "bs1")
                        evac(s1T[:, blk * 512:(blk + 1) * 512], ps1)
                    else:  # q or k: silu + sum-of-squares, 4 m-tiles per psum
                        w = wq if kind == "q" else wk
                        bn = "bq" if kind == "q" else "bk"
                        full = q_full if kind == "q" else k_full
                        ss = ss_q if kind == "q" else ss_k
                        g = blk * 4
                        pqk = ps_proj.tile([128, 4, D], F32, tag="proj", name="pqk")
                        for h in range(4):
                            for c in range(EC):
                                nc.tensor.matmul(
                                    pqk[:, h, :],
                                    xt[:, c, h * 128:(h + 1) * 128],
                                    w[:, c, :], start=(c == 0),
                                    stop=(c == EC - 1 and not has_bias[bn]))
                            if has_bias[bn]:
                                bias_mm(pqk[:, h, :], bn)
                        sig = sigp.tile([128, 4, D], BF, tag="sig", name="sigqk")
                        nc.scalar.activation(sig, pqk, AF.Tanh)
                        # silu(2x') = x'*(tanh(x')+1), x' = halved pre-act
                        nc.vector.scalar_tensor_tensor(
                            out=full[:, g:g + 4, :], in0=sig, scalar=1.0,
                            in1=pqk, op0=ALU.add, op1=ALU.mult)
                        scr = sigp.tile([128, 4, D], F32, tag="scr", name="scr")
                        nc.vector.tensor_mul(scr, full[:, g:g + 4, :], full[:, g:g + 4, :])
                        nc.vector.reduce_sum(
                            ss[:, g:g + 4].rearrange("p (a b) -> p a b", b=1),
                            scr, axis=mybir.AxisListType.X)

            # Order keeps ACT table-set usage monotonic (sigmoid -> sqrt ->
            # exp) while starting the critical path (hk -> kT -> scores) first.
            process(hk_d, 4, "k")
            process(hq_d, 2, "q")
            process(hv_d, 4, "v")
            process(hs_d, 2, "s")

            # shortcut (transposed): tsc = tanh(halved shortcut pre-act)
            for i5 in range(R // 512):
                ps2 = ps_mini.tile([64, 512], F32, tag="mini", name="ps2")
                nc.tensor.matmul(ps2, ws2, s1T[:, i5 * 512:(i5 + 1) * 512],
                                 start=True, stop=not has_bias["bs2"])
                biasT_mm(ps2, "bs2")
                nc.scalar.activation(tsc[:, i5 * 512:(i5 + 1) * 512], ps2, AF.Tanh)

            # l2-normalize q (and fold the 1/sqrt(D) score scale) and k:
            # rs_q = 1/sqrt(64*ss) = 0.125/sqrt(ss); sqrt on ACT, recip on DVE.
            lnq = sigp.tile([128, RT], F32, tag="lnq", name="lnq")
            nc.scalar.activation(lnq, ss_q, AF.Sqrt, scale=64.0, bias=eps_l2)
            nc.vector.reciprocal(rs_q, lnq)
            lnk = sigp.tile([128, KT], F32, tag="lnk", name="lnk")
            nc.scalar.activation(lnk, ss_k, AF.Sqrt, bias=eps_l2)
            nc.vector.reciprocal(rs_k, lnk)
            for t in range(RT):
                nc.vector.tensor_scalar_mul(qn[:, t, :], q_full[:, t, :],
                                            rs_q[:, t:t + 1])
            for t in range(KT):
                nc.vector.tensor_scalar_mul(kn[:, t, :], k_full[:, t, :],
                                            rs_k[:, t:t + 1])
            for t in range(RT):
                pt = ps_mini.tile([64, 128], BF, tag="mini", name="ptq")
                nc.tensor.transpose(pt, qn[:, t, :], ident)
                evac(qT2[0:64, t * 128:(t + 1) * 128], pt)
            for t in range(KT):
                pt = ps_mini.tile([64, 128], BF, tag="mini", name="ptk")
                nc.tensor.transpose(pt, kn[:, t, :], ident)
                evac(kT[:, t * 128:(t + 1) * 128], pt)
            # duplicate qT into partitions 64:128 and pack kT by jt parity so
            # score matmul pairs can run concurrently on disjoint PE row groups
            nc.sync.dma_start(out=qT2[64:128, :], in_=qT2[0:64, :])
            kTv = kT.rearrange("p (u two f) -> p u two f", two=2, f=128)
            nc.vector.tensor_copy(kT2[0:64, :, :], kTv[:, :, 0, :])
            nc.sync.dma_start(out=kT2[64:128, :, :], in_=kTv[:, :, 1, :])

        # ---------------- Phase B: attention ----------------
        with tc.tile_pool(name="bpool", bufs=2) as bpool, \
             tc.tile_pool(name="ps_e", bufs=2, space="PSUM") as ps_e, \
             tc.tile_pool(name="ps_sm", bufs=4, space="PSUM") as ps_sm:

            for u in range(KT // 2):
                peA = ps_e.tile([128, R], F32, tag="e", name="peA")
                peB = ps_e.tile([128, R], F32, tag="e", name="peB")
                for i5 in range(R // 512):
                    nc.tensor.matmul(peA[:, i5 * 512:(i5 + 1) * 512],
                                     kT2[0:64, u, :],
                                     qT2[0:64, i5 * 512:(i5 + 1) * 512],
                                     start=True, stop=True, tile_position=(0, 0))
                    nc.tensor.matmul(peB[:, i5 * 512:(i5 + 1) * 512],
                                     kT2[64:128, u, :],
                                     qT2[64:128, i5 * 512:(i5 + 1) * 512],
                                     start=True, stop=True, tile_position=(64, 0))
                nc.scalar.activation(eT[:, 2 * u, :], peA, AF.Exp)
                nc.scalar.activation(eT[:, 2 * u + 1, :], peB, AF.Exp)

            for ib in range(R // 512):
                pa = ps_sm.tile([64, 512], F32, tag="sp", name="pa")
                for jt in range(KT):
                    nc.tensor.matmul(pa, v1[:, jt, :],
                                     eT[:, jt, ib * 512:(ib + 1) * 512],
                                     start=(jt == 0), stop=(jt == KT - 1))
                # sum over d of attn_un^2, landed as per-partition columns so
                # the rmsnorm scale applies at the final evacuation.
                sq = bpool.tile([64, 512], BF, tag="sq", name="sq")
                nc.scalar.activation(sq, pa, AF.Square)
                pr = ps_sm.tile([1, 512], F32, tag="sp", name="pr")
                nc.tensor.matmul(pr, ones64, sq, start=True, stop=True)
                ssr = bpool.tile([1, 512], BF, tag="ssr", name="ssr")
                nc.vector.tensor_copy(ssr, pr)
                psc = ps_sm.tile([128, 4, 2], BF, tag="sp", name="psc")
                for tt in range(4):
                    nc.tensor.transpose(psc[:, tt, 0:1],
                                        ssr[:, tt * 128:(tt + 1) * 128], ones1)
                nc.vector.tensor_copy(ms_cols[:, ib * 4:(ib + 1) * 4],
                                      psc[:, :, 0])
                # yT' = attn_un*(tanh+1) = 2*attn_un*shortcut (2 folded into Wo)
                yT = bpool.tile([64, 512], BF, tag="yT", name="yT")
                nc.vector.scalar_tensor_tensor(
                    out=yT, in0=tsc[:, ib * 512:(ib + 1) * 512], scalar=1.0,
                    in1=pa, op0=ALU.add, op1=ALU.mult)

                # rs for this i-block; rmsnorm scale applied at final evac
                nrm = bpool.tile([128, 4], F32, tag="nrm", name="nrm")
                nc.scalar.activation(nrm, ms_cols[:, ib * 4:(ib + 1) * 4],
                                     AF.Sqrt, scale=1.0 / D, bias=eps_rms128)
                nc.vector.reciprocal(rs_cols[:, ib * 4:(ib + 1) * 4], nrm)
                for tt in range(4):
                    g = ib * 4 + tt
                    po = ps_sm.tile([128, D], F32, tag="sp", name="po")
                    nc.tensor.matmul(po, yT[:, tt * 128:(tt + 1) * 128],
                                     wo, start=True, stop=not has_bias["bo"])
                    bias_mm(po, "bo")
                    nc.vector.tensor_scalar_mul(out_sb[:, g, :], po,
                                                rs_cols[:, g:g + 1])
                nc.sync.dma_start(
                    out=out_d.ap().rearrange("(t p) n -> p t n", p=128)[
                        :, ib * 4:(ib + 1) * 4, :],
                    in_=out_sb[:, ib * 4:(ib + 1) * 4, :],
                )

    nc.compile()
    return nc


_CACHED = None


def kernel(**inputs):
    global LAST, _CACHED
    inp = {k: np.asarray(v) for k, v in inputs.items()}

    bias_map = {"bq": "bq", "bk": "bk", "ba1": "ba1", "ba2": "ba2",
                "bs1": "bs1", "bs2": "bs2", "bo": "bo"}
    has_bias = {k: bool(np.any(inp[v])) for k, v in bias_map.items()}
    has_bias["bvb"] = bool(np.any(inp["bv"]) or np.any(inp["bb"]))

    key = tuple(sorted(has_bias.items()))
    if _CACHED is None or _CACHED[0] != key:
        _CACHED = (key, _build(has_bias))
    nc = _CACHED[1]

    bf = lambda x: np.ascontiguousarray(x.astype(BF16))
    f32 = lambda x: np.ascontiguousarray(x.astype(np.float32))
    # Gate pre-activations are halved on the host so sigmoid(x)=0.5*tanh(x/2)+0.5
    # and silu(x)=x*sigmoid(x) reduce to tanh + one scalar_tensor_tensor op.
    # The resulting global factor 2 on v1/attn cancels in rmsnorm; the factor 2
    # from the shortcut gate is folded into Wo (with g_rms).
    wo_fold = 0.5 * inp["g_rms"][:, None] * inp["Wo"]
    weights = {
        "wq": bf(0.5 * inp["Wq"]), "wk": bf(0.5 * inp["Wk"]),
        "wvb": bf(0.5 * np.concatenate([inp["Wv"], inp["Wb"]], axis=1)),
        "wa1": bf(inp["Wa1"]), "ws1": bf(inp["Ws1"]),
        "wa2": bf(0.5 * inp["Wa2"]), "ws2": bf(0.5 * inp["Ws2"]),
        "wo": bf(wo_fold),
    }
    if has_bias["bq"]:
        weights["bq"] = bf(0.5 * inp["bq"][None, :])
    if has_bias["bk"]:
        weights["bk"] = bf(0.5 * inp["bk"][None, :])
    if has_bias["bvb"]:
        weights["bvb"] = bf(0.5 * np.concatenate([inp["bv"], inp["bb"]])[None, :])
    if has_bias["ba1"]:
        weights["ba1"] = bf(inp["ba1"][None, :])
    if has_bias["ba2"]:
        weights["ba2"] = bf(0.5 * inp["ba2"][None, :])
    if has_bias["bs1"]:
        weights["bs1"] = bf(inp["bs1"][None, :])
    if has_bias["bs2"]:
        weights["bs2"] = bf(0.5 * inp["bs2"][None, :])
    if has_bias["bo"]:
        weights["bo"] = bf(inp["bo"][None, :])

    in_maps = []
    for c in range(NCORES):
        b, h = c // 2, c % 2
        m = dict(weights)
        m["hq"] = f32(inp["hidden_query"][b, h * R:(h + 1) * R])
        m["hk"] = f32(inp["hidden_key"][b])
        m["hv"] = f32(inp["hidden_value"][b])
        m["hs"] = f32(inp["hidden_shortcut"][b, h * R:(h + 1) * R])
        in_maps.append(m)

    LAST = run_bass_kernel_spmd(nc, in_maps, core_ids=list(range(NCORES)))

    out = np.empty((B, L, D), np.float32)
    for c in range(NCORES):
        b, h = c // 2, c % 2
        out[b, h * R:(h + 1) * R] = LAST.results[c]["out"]
    return out


if __name__ == "__main__":
    rng = np.random.default_rng(0)
    fake = {}
    fake["hidden_query"] = rng.standard_normal((B, L, E), dtype=np.float32)
    fake["hidden_key"] = rng.standard_normal((B, L, E), dtype=np.float32)
    fake["hidden_value"] = rng.standard_normal((B, L, E), dtype=np.float32)
    fake["hidden_shortcut"] = rng.standard_normal((B, L, E), dtype=np.float32)
    for n, s in [("Wq", (E, D)), ("Wk", (E, D)), ("Wv", (E, D)), ("Wa1", (E, 32)),
                 ("Wa2", (32, D)), ("Wb", (E, D)), ("Ws1", (E, 32)), ("Ws2", (32, D)),
                 ("Wo", (D, D))]:
        fake[n] = rng.standard_normal(s, dtype=np.float32) * 0.05
    for n, s in [("bq", D), ("bk", D), ("bv", D), ("ba1", 32), ("ba2", D),
                 ("bb", D), ("bs1", 32), ("bs2", D), ("bo", D)]:
        fake[n] = np.zeros(s, np.float32)
    fake["g_rms"] = np.ones(D, np.float32)
    o = kernel(**fake)
    print("ran:", o.shape, o.dtype, np.abs(o).max())


# revision 36
# speedup vs baseline: 1.0228x; 1.0228x over previous
"""Trainium2 Bass kernel for gated single-head attention (B=4, L=2048, E=512, D=64).

Sharding: data-parallel over 8 cores; core c handles batch b=c//2, query-row
half h=c%2 (1024 query rows). k/v are computed per-core for the full batch
(duplicated across the pair) since softmax needs all 2048 keys.

Math restructuring (validated in numpy against the jax reference, ~4.8e-3 rel):
  - q,k are L2-normalized so scores lie in [-1/8, 1/8]: softmax needs no
    max-subtraction; exp() is applied directly to the transposed score tile.
  - softmax denominator Z is skipped entirely: rmsnorm is invariant to a
    per-row uniform scale (only the eps semantics shift, ~1e-5 effect).
  - rsqrt computed as exp(-0.5*ln(x)) so only two ACT table sets are used
    (sigmoid_and_others + natural_log_exp_and_others).
  - g_rms is folded into Wo on the host; silu(x) = x*sigmoid(x) on DVE.
Compute dtype is bf16 (PE runs 4x slower on f32), accumulation f32 in PSUM.
"""

import os
import sys

import numpy as np

try:
    import concourse.bass as bass
except ImportError:  # staged container path
    sys.path.insert(0, "/opt/trn_rl_repo")
    import concourse.bass as bass

import ml_dtypes
from contextlib import ExitStack

import concourse.bacc as bacc
import concourse.tile as tile
from concourse import mybir
from concourse.bass_utils import run_bass_kernel_spmd
from concourse.masks import make_identity

BF16 = ml_dtypes.bfloat16
F32 = mybir.dt.float32
BF = mybir.dt.bfloat16
AF = mybir.ActivationFunctionType
ALU = mybir.AluOpType

B, L, E, D = 4, 2048, 512, 64
NCORES = 8
R = L // 2          # 1024 query rows per core
RT = R // 128       # 8 query m-tiles per core
KT = L // 128       # 16 kv m-tiles per core
EC = E // 128       # 4 contraction chunks
EPS_RMS = 1e-6
EPS_L2 = 1e-24

LAST = None  # BassKernelResults of the most recent run (for test harness)


def _build(has_bias):
    """Build the per-core SPMD program. has_bias: dict of bool flags."""
    nc = bacc.Bacc(
        "TRN2",
        target_bir_lowering=False,
        debug=False,
        enable_asserts=False,
        num_devices=NCORES,
    )

    hq_d = nc.dram_tensor("hq", [R, E], F32, kind="ExternalInput")
    hk_d = nc.dram_tensor("hk", [L, E], F32, kind="ExternalInput")
    hv_d = nc.dram_tensor("hv", [L, E], F32, kind="ExternalInput")
    hs_d = nc.dram_tensor("hs", [R, E], F32, kind="ExternalInput")
    wq_d = nc.dram_tensor("wq", [E, D], BF, kind="ExternalInput")
    wk_d = nc.dram_tensor("wk", [E, D], BF, kind="ExternalInput")
    wvb_d = nc.dram_tensor("wvb", [E, 2 * D], BF, kind="ExternalInput")
    wa1_d = nc.dram_tensor("wa1", [E, 32], BF, kind="ExternalInput")
    ws1_d = nc.dram_tensor("ws1", [E, 32], BF, kind="ExternalInput")
    wa2_d = nc.dram_tensor("wa2", [32, D], BF, kind="ExternalInput")
    ws2_d = nc.dram_tensor("ws2", [32, D], BF, kind="ExternalInput")
    wo_d = nc.dram_tensor("wo", [D, D], BF, kind="ExternalInput")
    bias_d = {}
    for name, n in [("bq", D), ("bk", D), ("bvb", 2 * D), ("ba1", 32),
                    ("ba2", D), ("bs1", 32), ("bs2", D), ("bo", D)]:
        if has_bias[name]:
            bias_d[name] = nc.dram_tensor(name, [1, n], BF, kind="ExternalInput")
    out_d = nc.dram_tensor("out", [R, D], F32, kind="ExternalOutput")

    with tile.TileContext(nc) as tc, ExitStack() as ctx:
        consts = ctx.enter_context(tc.tile_pool(name="consts", bufs=1))
        persist = ctx.enter_context(tc.tile_pool(name="persist", bufs=1))

        ident = consts.tile([128, 128], BF)
        make_identity(nc, ident)
        ones64 = consts.tile([64, 1], BF)
        nc.vector.memset(ones64, 1.0)
        ones1 = consts.tile([1, 1], BF)
        nc.vector.memset(ones1, 1.0)
        onec = consts.tile([128, D], BF)
        nc.vector.memset(onec, 1.0)
        eps_rms128 = consts.tile([128, 1], F32)
        nc.vector.memset(eps_rms128, EPS_RMS)
        eps_l2 = consts.tile([128, 1], F32)
        nc.vector.memset(eps_l2, EPS_L2)
        any_bias = any(has_bias.values())
        if any_bias:
            ones_row = consts.tile([1, 512], BF)
            nc.vector.memset(ones_row, 1.0)

        def load_w(d, n, nm):
            t = consts.tile([128, EC, n], BF, name=nm)
            nc.sync.dma_start(out=t, in_=d.ap().rearrange("(c p) n -> p c n", p=128))
            return t

        wq = load_w(wq_d, D, "wq_sb")
        wk = load_w(wk_d, D, "wk_sb")
        wvb = load_w(wvb_d, 2 * D, "wvb_sb")
        wa1 = load_w(wa1_d, 32, "wa1_sb")
        ws1 = load_w(ws1_d, 32, "ws1_sb")
        wa2 = consts.tile([32, D], BF)
        nc.sync.dma_start(out=wa2, in_=wa2_d.ap())
        ws2 = consts.tile([32, D], BF)
        nc.sync.dma_start(out=ws2, in_=ws2_d.ap())
        wo = consts.tile([64, D], BF)
        nc.sync.dma_start(out=wo, in_=wo_d.ap())
        bias_sb = {}
        for name, t in bias_d.items():
            n = t.shape[1]
            bt = consts.tile([1, n], BF, name=f"{name}_sb")
            nc.sync.dma_start(out=bt, in_=t.ap())
            bias_sb[name] = bt

        def bias_mm(psum, name, cols=None):
            """Add per-column bias b[1, n] to psum accumulation via K=1 matmul."""
            if name not in bias_sb:
                return False
            b = bias_sb[name]
            if cols is not None:
                b = b[:, cols[0]:cols[1]]
            nc.tensor.matmul(psum, ones_row[:, : psum.shape[0]], b.rearrange("o n -> o n"),
                             start=False, stop=True)
            return True

        def biasT_mm(psum, name):
            """Add per-row bias (transposed layouts): psum[r, m] += b[r]."""
            if name not in bias_sb:
                return False
            nc.tensor.matmul(psum, bias_sb[name], ones_row[:, : psum.free_size()],
                             start=False, stop=True)
            return True

        # persistent SBUF tensors
        q_full = persist.tile([128, RT, D], F32)
        k_full = persist.tile([128, KT, D], F32)
        ss_q = persist.tile([128, RT], F32)
        ss_k = persist.tile([128, KT], F32)
        rs_q = persist.tile([128, RT], F32)
        rs_k = persist.tile([128, KT], F32)
        ms_cols = persist.tile([128, RT], F32)
        rs_cols = persist.tile([128, RT], F32)
        qn = persist.tile([128, RT, D], BF)
        kn = persist.tile([128, KT, D], BF)
        qT2 = persist.tile([128, R], BF)   # rows 0:64 = qT, 64:128 = copy
        kT2 = persist.tile([128, KT // 2, 128], BF)  # even jt rows 0:64, odd 64:128
        vb_tanh = persist.tile([128, KT, 2 * D], BF)  # tanh(v'), tanh(b')
        v_full = persist.tile([128, KT, D], BF)
        v1 = persist.tile([128, KT, D], BF)
        a1T = persist.tile([32, L], BF)
        s1T = persist.tile([32, R], BF)
        tsc = persist.tile([64, R], BF)   # tanh of halved shortcut pre-act
        eT = persist.tile([128, KT, R], BF)
        out_sb = persist.tile([128, RT, D], F32)

        evac_ct = [0]

        def evac(dst, src):
            """Copy PSUM->SBUF alternating between ACT and DVE."""
            evac_ct[0] += 1
            if evac_ct[0] % 2:
                nc.scalar.copy(dst, src)
            else:
                nc.vector.tensor_copy(dst, src)

        # ---------------- Phase A: load, transpose, project ----------------
        with tc.tile_pool(name="loadp", bufs=3) as loadp, \
             tc.tile_pool(name="xtp", bufs=2) as xtp, \
             tc.tile_pool(name="sigp", bufs=3) as sigp, \
             tc.tile_pool(name="ps_tp", bufs=2, space="PSUM") as ps_tp, \
             tc.tile_pool(name="ps_proj", bufs=2, space="PSUM") as ps_proj, \
             tc.tile_pool(name="ps_pt", bufs=2, space="PSUM") as ps_pt, \
             tc.tile_pool(name="ps_mini", bufs=2, space="PSUM") as ps_mini:

            def load_and_transpose(src_d, nblk, blk):
                src = src_d.ap().rearrange("(b t p) e -> b p t e", p=128, t=4)
                nat = loadp.tile([128, 4, E], BF, tag="nat", name="nat")
                nc.gpsimd.dma_start(out=nat, in_=src[blk])
                xt = xtp.tile([128, EC, 512], BF, tag="xt", name="xt")
                for cc in range(EC // 2):
                    # two e-chunks of transposes share one psum bank (bf16)
                    ps = ps_tp.tile([128, 2, 512], BF, tag="tp", name="ps_t")
                    for ci in range(2):
                        c = 2 * cc + ci
                        for t in range(4):
                            nc.tensor.transpose(
                                ps[:, ci, t * 128:(t + 1) * 128],
                                nat[:, t, c * 128:(c + 1) * 128],
                                ident,
                            )
                    nc.vector.tensor_copy(xt[:, 2 * cc:2 * cc + 2, :], ps)
                return xt

            def process(src_d, nblk, kind):
                for blk in range(nblk):
                    xt = load_and_transpose(src_d, nblk, blk)

                    if kind == "v":
                        for u in range(2):
                            jt = blk * 4 + 2 * u
                            # two m-tiles' [v|beta] groups in one psum bank
                            pvb = ps_proj.tile([128, 2, 2 * D], F32, tag="proj", name="pvb")
                            for h in range(2):
                                for c in range(EC):
                                    nc.tensor.matmul(
                                        pvb[:, h, :],
                                        xt[:, c, (2 * u + h) * 128:(2 * u + h + 1) * 128],
                                        wvb[:, c, :], start=(c == 0),
                                        stop=(c == EC - 1 and not has_bias["bvb"]))
                                bias_mm(pvb[:, h, :], "bvb")
                            nc.scalar.activation(vb_tanh[:, jt:jt + 2, :], pvb, AF.Tanh)
                            # v = x'*(tanh(x')+1) with x' = (hv@Wv + bv)/2
                            nc.vector.scalar_tensor_tensor(
                                out=v_full[:, jt:jt + 2, :],
                                in0=vb_tanh[:, jt:jt + 2, :D], scalar=1.0,
                                in1=pvb[:, :, :D], op0=ALU.add, op1=ALU.mult)
                        # a1T for this block
                        pa1 = ps_pt.tile([32, 512], F32, tag="pt", name="pa1")
                        for c in range(EC):
                            nc.tensor.matmul(pa1, wa1[:, c, :], xt[:, c, :],
                                             start=(c == 0),
                                             stop=(c == EC - 1 and not has_bias["ba1"]))
                        biasT_mm(pa1, "ba1")
                        evac(a1T[:, blk * 512:(blk + 1) * 512], pa1)
                        # alpha for this block's 4 m-tiles, then v1 = v*alpha+beta
                        # (scaled by 2: v1' = v*(ta+1) + (tb+1), cancels in rmsnorm)
                        pa2 = ps_proj.tile([128, 4, D], F32, tag="proj", name="pa2")
                        for h in range(4):
                            jt = blk * 4 + h
                            nc.tensor.matmul(pa2[:, h, :],
                                             a1T[:, jt * 128:(jt + 1) * 128], wa2,
                                             start=True, stop=not has_bias["ba2"])
                            if has_bias["ba2"]:
                                bias_mm(pa2[:, h, :], "ba2")
                        alf = sigp.tile([128, 4, D], BF, tag="sig", name="alf")
                        nc.scalar.activation(alf, pa2, AF.Tanh)
                        for h in range(4):
                            jt = blk * 4 + h
                            t1 = sigp.tile([128, D], BF, tag="t1", name="t1")
                            nc.vector.scalar_tensor_tensor(
                                out=t1, in0=alf[:, h, :], scalar=1.0,
                                in1=v_full[:, jt, :], op0=ALU.add, op1=ALU.mult)
                            c1 = sigp.tile([128, D], BF, tag="c1", name="c1")
                            nc.gpsimd.tensor_add(c1, vb_tanh[:, jt, D:], onec)
                            nc.gpsimd.tensor_add(v1[:, jt, :], t1, c1)
                    elif kind == "s":
                        ps1 = ps_pt.tile([32, 512], F32, tag="pt", name="ps1")
                        for c in range(EC):
                            nc.tensor.matmul(ps1, ws1[:, c, :], xt[:, c, :],
                                             start=(c == 0),
                                             stop=(c == EC - 1 and not has_bias["bs1"]))
                        biasT_mm(ps1, "bs1")
                        evac(s1T[:, blk * 512:(blk + 1) * 512], ps1)
                    else:  # q or k: silu + sum-of-squares, 4 m-tiles per psum
                        w = wq if kind == "q" else wk
                        bn = "bq" if kind == "q" else "bk"
                        full = q_full if kind == "q" else k_full
                        ss = ss_q if kind == "q" else ss_k
                        g = blk * 4
                        pqk = ps_proj.tile([128, 4, D], F32, tag="proj", name="pqk")
                        for h in range(4):
                            for c in range(EC):
                                nc.tensor.matmul(
                                    pqk[:, h, :],
                                    xt[:, c, h * 128:(h + 1) * 128],
                                    w[:, c, :], start=(c == 0),
                                    stop=(c == EC - 1 and not has_bias[bn]))
                            if has_bias[bn]:
                                bias_mm(pqk[:, h, :], bn)
                        sig = sigp.tile([128, 4, D], BF, tag="sig", name="sigqk")
                        nc.scalar.activation(sig, pqk, AF.Tanh)
                        # silu(2x') = x'*(tanh(x')+1), x' = halved pre-act
                        nc.vector.scalar_tensor_tensor(
                            out=full[:, g:g + 4, :], in0=sig, scalar=1.0,
                            in1=pqk, op0=ALU.add, op1=ALU.mult)
                        scr = sigp.tile([128, 4, D], F32, tag="scr", name="scr")
                        nc.vector.tensor_mul(scr, full[:, g:g + 4, :], full[:, g:g + 4, :])
                        nc.vector.reduce_sum(
                            ss[:, g:g + 4].rearrange("p (a b) -> p a b", b=1),
                            scr, axis=mybir.AxisListType.X)

            # Order keeps ACT table-set usage monotonic (sigmoid -> sqrt ->
            # exp) while starting the critical path (hk -> kT -> scores) first.
            process(hk_d, 4, "k")
            process(hq_d, 2, "q")
            process(hv_d, 4, "v")
            process(hs_d, 2, "s")

            # shortcut (transposed): tsc = tanh(halved shortcut pre-act)
            for i5 in range(R // 512):
                ps2 = ps_mini.tile([64, 512], F32, tag="mini", name="ps2")
                nc.tensor.matmul(ps2, ws2, s1T[:, i5 * 512:(i5 + 1) * 512],
                                 start=True, stop=not has_bias["bs2"])
                biasT_mm(ps2, "bs2")
                nc.scalar.activation(tsc[:, i5 * 512:(i5 + 1) * 512], ps2, AF.Tanh)

            # l2-normalize q (and fold the 1/sqrt(D) score scale) and k:
            # rs_q = 1/sqrt(64*ss) = 0.125/sqrt(ss); sqrt on ACT, recip on DVE.
            lnq = sigp.tile([128, RT], F32, tag="lnq", name="lnq")
            nc.scalar.activation(lnq, ss_q, AF.Sqrt, scale=64.0, bias=eps_l2)
            nc.vector.reciprocal(rs_q, lnq)
            lnk = sigp.tile([128, KT], F32, tag="lnk", name="lnk")
            nc.scalar.activation(lnk, ss_k, AF.Sqrt, bias=eps_l2)
            nc.vector.reciprocal(rs_k, lnk)
            for t in range(RT):
                nc.vector.tensor_scalar_mul(qn[:, t, :], q_full[:, t, :],
                                            rs_q[:, t:t + 1])
            for t in range(KT):
                nc.vector.tensor_scalar_mul(kn[:, t, :], k_full[:, t, :],
                                            rs_k[:, t:t + 1])
            # Transpose qn/kn with D on partitions, packing both PE row-group
            # halves directly (qT duplicated, kT split by jt parity) so score
            # matmul pairs run concurrently on disjoint row groups.
            for t in range(RT):
                pt = ps_mini.tile([128, 128], BF, tag="mini", name="ptq")
                nc.tensor.transpose(pt[0:64, :], qn[:, t, :], ident)
                nc.tensor.transpose(pt[64:128, :], qn[:, t, :], ident,
                                    tile_position=(0, 64))
                evac(qT2[:, t * 128:(t + 1) * 128], pt)
            for u in range(KT // 2):
                pt = ps_mini.tile([128, 128], BF, tag="mini", name="ptk")
                nc.tensor.transpose(pt[0:64, :], kn[:, 2 * u, :], ident)
                nc.tensor.transpose(pt[64:128, :], kn[:, 2 * u + 1, :], ident,
                                    tile_position=(0, 64))
                evac(kT2[:, u, :], pt)

        # ---------------- Phase B: attention ----------------
        with tc.tile_pool(name="bpool", bufs=2) as bpool, \
             tc.tile_pool(name="ps_e", bufs=2, space="PSUM") as ps_e, \
             tc.tile_pool(name="ps_sm", bufs=4, space="PSUM") as ps_sm:

            for u in range(KT // 2):
                peA = ps_e.tile([128, R], F32, tag="e", name="peA")
                peB = ps_e.tile([128, R], F32, tag="e", name="peB")
                for i5 in range(R // 512):
                    nc.tensor.matmul(peA[:, i5 * 512:(i5 + 1) * 512],
                                     kT2[0:64, u, :],
                                     qT2[0:64, i5 * 512:(i5 + 1) * 512],
                                     start=True, stop=True, tile_position=(0, 0))
                    nc.tensor.matmul(peB[:, i5 * 512:(i5 + 1) * 512],
                                     kT2[64:128, u, :],
                                     qT2[64:128, i5 * 512:(i5 + 1) * 512],
                                     start=True, stop=True, tile_position=(64, 0))
                nc.scalar.activation(eT[:, 2 * u, :], peA, AF.Exp)
                nc.scalar.activation(eT[:, 2 * u + 1, :], peB, AF.Exp)

            for ib in range(R // 512):
                pa = ps_sm.tile([64, 512], F32, tag="sp", name="pa")
                for jt in range(KT):
                    nc.tensor.matmul(pa, v1[:, jt, :],
                                     eT[:, jt, ib * 512:(ib + 1) * 512],
                                     start=(jt == 0), stop=(jt == KT - 1))
                # sum over d of attn_un^2, landed as per-partition columns so
                # the rmsnorm scale applies at the final evacuation.
                sq = bpool.tile([64, 512], BF, tag="sq", name="sq")
                nc.scalar.activation(sq, pa, AF.Square)
                pr = ps_sm.tile([1, 512], F32, tag="sp", name="pr")
                nc.tensor.matmul(pr, ones64, sq, start=True, stop=True)
                ssr = bpool.tile([1, 512], BF, tag="ssr", name="ssr")
                nc.vector.tensor_copy(ssr, pr)
                psc = ps_sm.tile([128, 4, 2], BF, tag="sp", name="psc")
                for tt in range(4):
                    nc.tensor.transpose(psc[:, tt, 0:1],
                                        ssr[:, tt * 128:(tt + 1) * 128], ones1)
                nc.vector.tensor_copy(ms_cols[:, ib * 4:(ib + 1) * 4],
                                      psc[:, :, 0])
                # yT' = attn_un*(tanh+1) = 2*attn_un*shortcut (2 folded into Wo)
                yT = bpool.tile([64, 512], BF, tag="yT", name="yT")
                nc.vector.scalar_tensor_tensor(
                    out=yT, in0=tsc[:, ib * 512:(ib + 1) * 512], scalar=1.0,
                    in1=pa, op0=ALU.add, op1=ALU.mult)

                # rs for this i-block; rmsnorm scale applied at final evac
                nrm = bpool.tile([128, 4], F32, tag="nrm", name="nrm")
                nc.scalar.activation(nrm, ms_cols[:, ib * 4:(ib + 1) * 4],
                                     AF.Sqrt, scale=1.0 / D, bias=eps_rms128)
                nc.vector.reciprocal(rs_cols[:, ib * 4:(ib + 1) * 4], nrm)
                for tt in range(4):
                    g = ib * 4 + tt
                    po = ps_sm.tile([128, D], F32, tag="sp", name="po")
                    nc.tensor.matmul(po, yT[:, tt * 128:(tt + 1) * 128],
                                     wo, start=True, stop=not has_bias["bo"])
                    bias_mm(po, "bo")
                    nc.vector.tensor_scalar_mul(out_sb[:, g, :], po,
                                                rs_cols[:, g:g + 1])
                nc.sync.dma_start(
                    out=out_d.ap().rearrange("(t p) n -> p t n", p=128)[
                        :, ib * 4:(ib + 1) * 4, :],
                    in_=out_sb[:, ib * 4:(ib + 1) * 4, :],
                )

    nc.compile()
    return nc


_CACHED = None


def kernel(**inputs):
    global LAST, _CACHED
    inp = {k: np.asarray(v) for k, v in inputs.items()}

    bias_map = {"bq": "bq", "bk": "bk", "ba1": "ba1", "ba2": "ba2",
                "bs1": "bs1", "bs2": "bs2", "bo": "bo"}
    has_bias = {k: bool(np.any(inp[v])) for k, v in bias_map.items()}
    has_bias["bvb"] = bool(np.any(inp["bv"]) or np.any(inp["bb"]))

    key = tuple(sorted(has_bias.items()))
    if _CACHED is None or _CACHED[0] != key:
        _CACHED = (key, _build(has_bias))
    nc = _CACHED[1]

    bf = lambda x: np.ascontiguousarray(x.astype(BF16))
    f32 = lambda x: np.ascontiguousarray(x.astype(np.float32))
    # Gate pre-activations are halved on the host so sigmoid(x)=0.5*tanh(x/2)+0.5
    # and silu(x)=x*sigmoid(x) reduce to tanh + one scalar_tensor_tensor op.
    # The resulting global factor 2 on v1/attn cancels in rmsnorm; the factor 2
    # from the shortcut gate is folded into Wo (with g_rms).
    wo_fold = 0.5 * inp["g_rms"][:, None] * inp["Wo"]
    weights = {
        "wq": bf(0.5 * inp["Wq"]), "wk": bf(0.5 * inp["Wk"]),
        "wvb": bf(0.5 * np.concatenate([inp["Wv"], inp["Wb"]], axis=1)),
        "wa1": bf(inp["Wa1"]), "ws1": bf(inp["Ws1"]),
        "wa2": bf(0.5 * inp["Wa2"]), "ws2": bf(0.5 * inp["Ws2"]),
        "wo": bf(wo_fold),
    }
    if has_bias["bq"]:
        weights["bq"] = bf(0.5 * inp["bq"][None, :])
    if has_bias["bk"]:
        weights["bk"] = bf(0.5 * inp["bk"][None, :])
    if has_bias["bvb"]:
        weights["bvb"] = bf(0.5 * np.concatenate([inp["bv"], inp["bb"]])[None, :])
    if has_bias["ba1"]:
        weights["ba1"] = bf(inp["ba1"][None, :])
    if has_bias["ba2"]:
        weights["ba2"] = bf(0.5 * inp["ba2"][None, :])
    if has_bias["bs1"]:
        weights["bs1"] = bf(inp["bs1"][None, :])
    if has_bias["bs2"]:
        weights["bs2"] = bf(0.5 * inp["bs2"][None, :])
    if has_bias["bo"]:
        weights["bo"] = bf(inp["bo"][None, :])

    in_maps = []
    for c in range(NCORES):
        b, h = c // 2, c % 2
        m = dict(weights)
        m["hq"] = f32(inp["hidden_query"][b, h * R:(h + 1) * R])
        m["hk"] = f32(inp["hidden_key"][b])
        m["hv"] = f32(inp["hidden_value"][b])
        m["hs"] = f32(inp["hidden_shortcut"][b, h * R:(h + 1) * R])
        in_maps.append(m)

    LAST = run_bass_kernel_spmd(nc, in_maps, core_ids=list(range(NCORES)))

    out = np.empty((B, L, D), np.float32)
    for c in range(NCORES):
        b, h = c // 2, c % 2
        out[b, h * R:(h + 1) * R] = LAST.results[c]["out"]
    return out


if __name__ == "__main__":
    rng = np.random.default_rng(0)
    fake = {}
    fake["hidden_query"] = rng.standard_normal((B, L, E), dtype=np.float32)
    fake["hidden_key"] = rng.standard_normal((B, L, E), dtype=np.float32)
    fake["hidden_value"] = rng.standard_normal((B, L, E), dtype=np.float32)
    fake["hidden_shortcut"] = rng.standard_normal((B, L, E), dtype=np.float32)
    for n, s in [("Wq", (E, D)), ("Wk", (E, D)), ("Wv", (E, D)), ("Wa1", (E, 32)),
                 ("Wa2", (32, D)), ("Wb", (E, D)), ("Ws1", (E, 32)), ("Ws2", (32, D)),
                 ("Wo", (D, D))]:
        fake[n] = rng.standard_normal(s, dtype=np.float32) * 0.05
    for n, s in [("bq", D), ("bk", D), ("bv", D), ("ba1", 32), ("ba2", D),
                 ("bb", D), ("bs1", 32), ("bs2", D), ("bo", D)]:
        fake[n] = np.zeros(s, np.float32)
    fake["g_rms"] = np.ones(D, np.float32)
    o = kernel(**fake)
    print("ran:", o.shape, o.dtype, np.abs(o).max())
